# revision 46
# baseline (speedup 1.0000x reference)
"""Depthwise causal Conv1d (k=4) + SiLU on 8 Trainium2 NeuronCores.

Problem: x [4, 4096, 2048] f32, w [2048, 4] f32,
out[b, t, d] = silu(sum_j w[d, j] * x[b, t - 3 + j, d])   (zero-padded left).

Sharding: 8 cores = 4 batches x 2 channel-halves. Depthwise conv is
independent per channel, so channel sharding needs no halo exchange.

Layout: each core receives its shard host-transposed to [channels, time]
(channels on SBUF partitions). The per-channel weight w[d, j] is then a
per-partition scalar and the causal time shifts are free-dim AP offsets
into one loaded tile.

The kernel is HBM-bound: 8.4 MB in + 8.4 MB out per core in fp16 is
~47 us at the ~22 GB/s-per-queue x 16-queue DMA roof, so the span is
set by how tightly the queues pack. Structure that matters:
 - The whole x shard (66 KB/partition) is prefetched with dependency-
   free DMAs emitted in consumption order; early blocks on the SyncE
   HWDGE ring, x3/x5/x7 triggered from the ACT stream (two rings keep
   more descriptors in flight, and the queues round-robin rings per
   DESCRIPTOR, so fat 8 KB x rows dominate the 2 KB store rows).
 - The identity block and the fp16 weight table ride in the first 160
   columns of x block 0's rows: as standalone loads their tiny rows
   would each wait a full round-robin turn behind fat x rows (~13 us
   observed); packed into x0a they land with the first x data (~10 us).
 - Work split by measured rates (PE 7.4 us/block, DVE 12 us/block):
   blocks {0,2,3,5,6} + chunk 0 of 7 on the TensorEngine as diag(w_j)
   matmuls accumulating 4 taps in PSUM (diags built on-chip from the
   identity x per-partition weight, interleaved between DVE chunks so
   the elementwise path starts at x1 arrival); blocks {1,4} + the rest
   of 7 elementwise on DVE: 4 tensor_scalar products (2x fp16 mode,
   ~400 G elem/s) + pair-packed adds (1x, ~235 G elem/s).
   scalar_tensor_tensor would fuse mul+add but runs ~4x slower than
   tensor_scalar -- do not use it.
 - ACT does only the SiLUs; stores issue on GpSimd (SWDGE) so a store
   blocked on its SiLU never head-of-line-blocks load issue.

Precision: x, w and the output are fp16 (halves HBM traffic both ways
and enables the fast DVE tensor_scalar mode); the DVE add tree stays
fp16, PE accumulates fp32 in PSUM; SiLU computes fp32-internally on
ACT. End-to-end relative error ~5e-4.
"""

import sys
import types

import numpy as np

import concourse.bass as bass
import concourse.bacc as bacc
import concourse.mybir as mybir
from concourse.tile import TileContext
from concourse.bass_utils import run_bass_kernel_spmd


def _ensure_ntff_hook():
    """bass_utils imports antenv.axon_hooks when BASS_TRACE is set; that
    module is absent on this image. Install a shim so tracing works when
    possible and degrades gracefully (instead of crashing) when not."""
    try:
        import antenv.axon_hooks  # noqa: F401

        return
    except ImportError:
        pass
    try:
        import antenv

        hook = None
        try:
            if "/root/.axon_site" not in sys.path:
                sys.path.insert(0, "/root/.axon_site")
            from trn_agent_boot.trn_boot import _ntff_profile_via_ctypes

            hook = _ntff_profile_via_ctypes("/opt/axon/libaxon_pjrt.so")
        except Exception:
            hook = None
        mod = types.ModuleType("antenv.axon_hooks")
        mod._hook = hook
        mod.get_axon_ntff_profile_hook = lambda: mod._hook
        mod.set_axon_ntff_profile_hook = lambda h: setattr(mod, "_hook", h)
        sys.modules["antenv.axon_hooks"] = mod
        antenv.axon_hooks = mod
    except Exception:
        pass


_ensure_ntff_hook()

B, L, D = 4, 4096, 2048
K = 4
PAD = K - 1
N_CORES = 8
DH = D // 2            # channels per core
NBLK = DH // 128       # 128-partition channel blocks per core

OFF = 160              # appendix cols in block 0: ident [0,128) + wt [128,160)
ROWW = 4288            # DRAM row stride (fp16 elems): 64B-aligned rows

MID_DT = mybir.dt.float16
PE_BLKS = (0, 2, 3, 5, 6)   # full blocks on the TensorEngine
EL_BLKS = (1, 4)            # full blocks elementwise on DVE
SPLIT_BLK = 7               # [0,1024) on PE, [1024,4096) on DVE
CW = 2048                   # elementwise chunk width
PQ = 1024                   # PE PSUM chunk width

_cache = {}


def _build_bass():
    nc = bacc.Bacc()
    xt = nc.dram_tensor("xt", [DH, ROWW], MID_DT, kind="ExternalInput")
    ot = nc.dram_tensor("ot", [DH, L], MID_DT, kind="ExternalOutput")
    f32 = mybir.dt.float32

    with TileContext(nc) as tc:
        with tc.tile_pool(name="pool", bufs=2) as pool, \
             tc.tile_pool(name="psum", bufs=2, space="PSUM") as psum_pool:
            # tiny warmup Silu forces the silu activation-table load
            # right away (Copy, used by the diag builds, is in every set)
            warm = pool.tile([128, 2], MID_DT, tag="warm", bufs=1)
            nc.vector.memset(warm[:], 0.0)
            nc.scalar.activation(warm[:], warm[:], mybir.ActivationFunctionType.Silu)

            # x tiles; block 0's tile also holds the appendix (ident+wt).
            # Block 0 is loaded in two halves and the halves get SEPARATE
            # tiles: matmuls reading a tile that a DMA is concurrently
            # writing run at half rate (measured 427 ns vs 216 ns per
            # 512-col matmul), so never read a tile mid-write.
            x_tiles = {}
            for blk in range(NBLK):
                cols = (OFF + L // 2 + PAD + 1) if blk == 0 else (L + PAD + 1)
                x_tiles[blk] = pool.tile(
                    [128, cols], MID_DT, tag=f"x{blk}", bufs=1, name=f"x{blk}"
                )
            # second half of block 0 (padded cols [2048, 4099)), 3-col halo
            x0b = pool.tile(
                [128, L // 2 + PAD + 1], MID_DT, tag="x0b", bufs=1, name="x0b"
            )
            x0 = x_tiles[0]
            idt = x0[:, 0:128]
            w16 = x0[:, 128:OFF]          # [128, NBLK*K] fp16 weights
            # tensor_scalar requires an f32 scalar operand: one tiny
            # DVE copy converts the packed fp16 table after x0a lands
            wf = pool.tile([128, NBLK * K], f32, tag="wf", bufs=1)

            def wj(blk, j):
                return wf[:, blk * K + j : blk * K + j + 1]

            def load(blk, eng, lo, hi):
                r0 = blk * 128
                d0 = 0 if blk == 0 else OFF
                return eng.dma_start(
                    out=x_tiles[blk][:, lo:hi], in_=xt[r0 : r0 + 128, d0 + lo : d0 + hi]
                )

            # ALL x loads on the SyncE ring in strict consumption order
            # (block 0 quartered and block 1 halved so the PE/DVE starts
            # aren't gated on a full-block transfer; x0's first quarter
            # leads with the appendix). One ring = one FIFO: the
            # scheduler cannot reorder same-engine DMAs and the ~4-deep
            # trigger ring self-paces the tail, so arrivals exactly
            # track consumption.
            load(0, nc.sync, 0, OFF + L // 2 + PAD)
            nc.sync.dma_start(
                out=x0b[:, 0 : L // 2 + PAD],
                in_=xt[0:128, OFF + L // 2 : OFF + L + PAD],
            )
            for blk in range(1, NBLK):
                load(blk, nc.sync, 0, L + PAD)

            nc.vector.tensor_copy(wf[:], w16)

            # diag(w_j) stationary operands for the PE path, built on-chip:
            # diag[p, m] = ident[p, m] * w[p, j]. Blocks 0/2 up front (PE
            # needs them first); later blocks' builds are interleaved
            # between DVE chunks so the elementwise path starts at x1.
            diags = {}

            def build_diag(blk):
                dg = pool.tile(
                    [128, K * 128], MID_DT, tag=f"dg{blk}", bufs=1, name=f"dg{blk}"
                )
                diags[blk] = dg
                for j in range(K):
                    nc.vector.tensor_scalar_mul(
                        dg[:, j * 128 : (j + 1) * 128], idt, wj(blk, j)
                    )

            build_diag(0)
            build_diag(2)

            def pe_chunk(blk, h0):
                r0 = blk * 128
                x, dg = x_tiles[blk], diags[blk]
                xo = OFF if blk == 0 else 0
                if blk == 0 and h0 >= L // 2:
                    x, xo = x0b, -(L // 2)
                ps = psum_pool.tile([128, PQ], f32, tag="ps", bufs=4)
                for j in range(K):
                    lw = dg[:, j * 128 : (j + 1) * 128]
                    for c in range(PQ // 512):
                        nc.tensor.matmul(
                            ps[:, c * 512 : (c + 1) * 512],
                            lw,
                            x[:, xo + h0 + c * 512 + j : xo + h0 + c * 512 + j + 512],
                            start=(j == 0),
                            stop=(j == K - 1),
                        )
                o = pool.tile([128, PQ], MID_DT, tag="ope", bufs=4)
                silu = nc.scalar.activation(
                    o[:], ps[:], mybir.ActivationFunctionType.Silu
                )
                nc.gpsimd.dma_start(out=ot[r0 : r0 + 128, h0 : h0 + PQ], in_=o[:])
                return silu

            def el_chunk(blk, t0, tl):
                r0 = blk * 128
                x = x_tiles[blk]
                xo = OFF if blk == 0 else 0
                # qe holds the even-shift products [q0 | q2], qo the odd
                # [q1 | q3], each one contiguous [128, 2, tl] tile so both
                # pair-adds run as a single tensor_tensor op. Products are
                # shift-rebased: q_j[:, t] = w_j * x[:, t + j].
                qe = pool.tile([128, 2, CW], MID_DT, tag="qe", bufs=2)
                qo = pool.tile([128, 2, CW], MID_DT, tag="qo", bufs=2)
                t0 = xo + t0
                nc.vector.tensor_scalar_mul(
                    qe[:, 0, 0:tl], x[:, t0 : t0 + tl], wj(blk, 0)
                )
                nc.vector.tensor_scalar_mul(
                    qo[:, 0, 0:tl], x[:, t0 + 1 : t0 + 1 + tl], wj(blk, 1)
                )
                nc.vector.tensor_scalar_mul(
                    qe[:, 1, 0:tl], x[:, t0 + 2 : t0 + 2 + tl], wj(blk, 2)
                )
                nc.vector.tensor_scalar_mul(
                    qo[:, 1, 0:tl], x[:, t0 + 3 : t0 + 3 + tl], wj(blk, 3)
                )
                t0 -= xo
                if tl == CW:
                    nc.vector.tensor_add(qe[:, :, :], qe[:, :, :], qo[:, :, :])
                else:
                    nc.vector.tensor_add(
                        qe[:, 0, 0:tl], qe[:, 0, 0:tl], qo[:, 0, 0:tl]
                    )
                    nc.vector.tensor_add(
                        qe[:, 1, 0:tl], qe[:, 1, 0:tl], qo[:, 1, 0:tl]
                    )
                nc.vector.tensor_add(qe[:, 0, 0:tl], qe[:, 0, 0:tl], qe[:, 1, 0:tl])
                o = pool.tile([128, CW], MID_DT, tag="oel", bufs=4)
                nc.scalar.activation(
                    o[:, 0:tl], qe[:, 0, 0:tl], mybir.ActivationFunctionType.Silu
                )
                # stores kept at 2 KB DRAM rows: the queues round-robin
                # rings per DESCRIPTOR, so fat store rows would steal
                # proportionally more bandwidth from the in-flight loads
                for s0 in range(0, tl, 1024):
                    nc.gpsimd.dma_start(
                        out=ot[r0 : r0 + 128, t0 + s0 : t0 + s0 + min(1024, tl - s0)],
                        in_=o[:, s0 : s0 + min(1024, tl - s0)],
                    )

            for blk in [0, 1, 2, 3, 4, 5, 6, 7]:
                if blk in PE_BLKS:
                    for chunk in range(L // PQ):
                        pe_chunk(blk, chunk * PQ)
                elif blk in EL_BLKS:
                    el_chunk(blk, 0, CW)
                    if blk == 1:
                        build_diag(3)
                        build_diag(5)
                    else:
                        build_diag(6)
                        build_diag(SPLIT_BLK)
                    el_chunk(blk, CW, CW)
                else:  # SPLIT_BLK: [0,1024) on PE, the rest elementwise.
                    # el chunks emitted FIRST so their SiLUs sit before
                    # the PE chunk's in the in-order ACT stream (the PE
                    # chunk is the TensorEngine's last work and would
                    # otherwise serialize the whole tail behind it).
                    el_chunk(blk, 1024, CW)
                    el_chunk(blk, 1024 + CW, L - 1024 - CW)
                    pe_chunk(blk, 0)
    nc.compile()
    return nc


def _shard_inputs(x, w):
    in_maps = []
    for core in range(N_CORES):
        b, half = divmod(core, 2)
        d0 = half * DH
        xt = np.zeros((DH, ROWW), dtype=np.float16)
        xt[:, OFF + PAD : OFF + PAD + L] = x[b, :, d0 : d0 + DH].T.astype(np.float16)
        # appendix in block 0's rows: identity then the fp16 weight table
        # w16[p, blk*K + j] = w[blk*128 + p, j]
        xt[0:128, 0:128] = np.eye(128, dtype=np.float16)
        w_sh = w[d0 : d0 + DH].reshape(NBLK, 128, K)
        xt[0:128, 128:OFF] = (
            w_sh.transpose(1, 0, 2).reshape(128, NBLK * K).astype(np.float16)
        )
        in_maps.append({"xt": np.ascontiguousarray(xt)})
    return in_maps


def kernel(x, w):
    x = np.asarray(x, dtype=np.float32)
    w = np.asarray(w, dtype=np.float32)
    assert x.shape == (B, L, D) and w.shape == (D, K)

    if "nc" not in _cache:
        _cache["nc"] = _build_bass()
    nc = _cache["nc"]

    in_maps = _shard_inputs(x, w)
    res = None
    for attempt in range(3):
        try:
            res = run_bass_kernel_spmd(nc, in_maps, core_ids=list(range(N_CORES)))
            break
        except Exception:
            if attempt == 2:
                raise
    _cache["last_results"] = res

    out = np.empty((B, L, D), dtype=np.float32)
    for core in range(N_CORES):
        b, half = divmod(core, 2)
        d0 = half * DH
        out[b, :, d0 : d0 + DH] = res.results[core]["ot"].T.astype(np.float32)
    return out
